# revision 15
# baseline (speedup 1.0000x reference)
"""Routed MoE classifier head for Trainium2 (8 NeuronCores, SPMD).

The reference computes all 8 experts densely and selects; here each sample is
routed to exactly one expert (expert e -> core e, padded to a common S).
Each core runs a dense 2-layer MLP (768 -> relu 384 -> 8) over its expert's
samples, with x pre-transposed so the contraction dim D lands on SBUF
partitions.

Layer 1 uses fp8(e4m3) matmuls in DoubleRow perf mode (contraction 256 per
pass, 2 moving columns/cycle — 2x the bf16 rate) with full error
compensation:

    16*(x @ W1) = x8 @ W8 + rx8 @ W8 + x8 @ rW8        (one PSUM group)

where  x8 = e4m3(x),  rx8 = e4m3(x - x8),  W8 = e4m3(16*W1),
rW8 = e4m3(16*W1 - W8).  The dropped term rx@rW is ~0.2% rms.  The PSUM
group is rescaled for free by the relu epilogue: h = relu(PSUM/16 + b1)
(ACT computes func(in*scale + bias)).  This is bf16-level accuracy at 1.5x
the bf16 matmul throughput: layer 1 costs 13.5 PE-cycles/sample instead of
18 (l2, in bf16, adds 3).

DMA cost is identical to bf16 (two e4m3 tensors = one bf16 tensor).
Output y^T [8, S] is scattered back on the host.
"""

import ml_dtypes
import numpy as np

import concourse.bass as bass
import concourse.mybir as mybir
from concourse.tile import TileContext
from concourse.bass_utils import run_bass_kernel_spmd

P = 128
D = 768
H = 384
C = 8
E = 8
NCORES = 8
DPAIR = D // (2 * P)  # 3 k-tile pairs (contraction 256 each)
HBLK = H // P  # 3
CHUNK = 512  # compute chunk (one PSUM bank of fp32)
XGRAN = 1536  # x DMA granularity (samples per load; multiple of CHUNK)
YGRAN = 2048  # y DMA granularity (samples per store)

F8 = ml_dtypes.float8_e4m3
WSCALE = 16.0

_program_cache = {}
last_results = None  # BassKernelResults of the most recent run (for test harness)


def _split_excess_waits(nc, max_waits=1):
    """The walrus build in this container only encodes one sem-wait per
    instruction; hoist extra waits onto NOPs inserted just before."""
    for blk in nc.main_func.blocks:
        insts = blk.instructions
        i = 0
        while i < len(insts):
            inst = insts[i]
            si = getattr(inst, "sync_info", None)
            if si is not None and si.on_wait and len(si.on_wait) > max_waits:
                waits = list(si.on_wait)
                extra, keep = waits[:-max_waits], waits[-max_waits:]
                nops = []
                for j in range(0, len(extra), max_waits):
                    nops.append(
                        mybir.InstNoOp(
                            name=f"{inst.name}-wsplit{j}",
                            engine=inst.engine,
                            bass_nofuse=True,
                            sync_info=mybir.SyncInfo(
                                on_wait=extra[j : j + max_waits], on_update=[]
                            ),
                        )
                    )
                inst.sync_info = mybir.SyncInfo(on_wait=keep, on_update=si.on_update)
                for k, nop in enumerate(nops):
                    nc.register_instruction(nop, overwrite=True)
                    insts.insert(i + k, nop)
                i += len(nops)
            i += 1
    return nc


def _spans2(total, lead, gran):
    """[(off, n), ...] covering `total`: leading spans from `lead`, then
    `gran`-sized spans (last one smaller)."""
    spans = []
    off = 0
    k = 0
    while off < total:
        n = min(lead[k] if k < len(lead) else gran, total - off)
        spans.append((off, n))
        off += n
        k += 1
    return spans


def _build_program(S):
    f32 = mybir.dt.float32
    bf16 = mybir.dt.bfloat16
    f8 = mybir.dt.float8e4
    relu = mybir.ActivationFunctionType.Relu
    add = mybir.AluOpType.add
    dr = mybir.MatmulPerfMode.DoubleRow

    nc = bass.Bass(enable_partition_id=False)
    x8t = nc.dram_tensor("x8t", [P, DPAIR, 2, S], f8, kind="ExternalInput")
    r8t = nc.dram_tensor("r8t", [P, DPAIR, 2, S], f8, kind="ExternalInput")
    w8t = nc.dram_tensor("w8t", [P, DPAIR, 2, H], f8, kind="ExternalInput")
    rw8t = nc.dram_tensor("rw8t", [P, DPAIR, 2, H], f8, kind="ExternalInput")
    w2t = nc.dram_tensor("w2t", [P, HBLK * C], bf16, kind="ExternalInput")
    bt = nc.dram_tensor("bt", [P, HBLK + 1], f32, kind="ExternalInput")
    yt = nc.dram_tensor("yt", [C, S], f32, kind="ExternalOutput")

    x_spans = _spans2(S, [CHUNK, CHUNK, CHUNK], XGRAN)

    with TileContext(nc) as tc:
        with (
            tc.tile_pool(name="const", bufs=1) as cpool,
            tc.tile_pool(name="xin", bufs=3) as xpool,
            tc.tile_pool(name="rin", bufs=3) as rpool,
            tc.tile_pool(name="hbuf", bufs=3) as hpool,
            tc.tile_pool(name="yout", bufs=2) as ypool,
            tc.tile_pool(name="psum1", bufs=6, space="PSUM") as pp1,
            tc.tile_pool(name="psum2", bufs=2, space="PSUM") as pp2,
        ):
            b_t = cpool.tile([P, HBLK + 1], f32)
            w2_t = cpool.tile([P, HBLK * C], bf16)
            w8_t = cpool.tile([P, DPAIR, 2, H], f8)
            rw8_t = cpool.tile([P, DPAIR, 2, H], f8)
            span_tiles = {}
            nxt = 0  # next span index to issue

            def load_x_span(eng):
                nonlocal nxt
                si = nxt
                nxt += 1
                off, n = x_spans[si]
                x_t = xpool.tile([P, DPAIR, 2, XGRAN], f8, name="x_t")
                r_t = rpool.tile([P, DPAIR, 2, XGRAN], f8, name="r_t")
                eng.dma_start(x_t[:, :, :, :n], x8t[:, :, :, off : off + n])
                eng.dma_start(r_t[:, :, :, :n], r8t[:, :, :, off : off + n])
                span_tiles[si] = (x_t, r_t)

            # Startup: chunk 0 is dp-outer, so its dependencies are needed in
            # dp order.  Interleave the per-dp pieces of {W8, x8} on sync and
            # {rW8, rx8} on scalar so pass 1 of each dp-block can start while
            # the correction operands for it are still in flight.  The PE
            # must never idle >1us once started (HAM warmup window resets).
            off0, n0 = x_spans[0]
            x0 = xpool.tile([P, DPAIR, 2, XGRAN], f8, name="x_t")
            r0 = rpool.tile([P, DPAIR, 2, XGRAN], f8, name="r_t")
            for dp in range(DPAIR):
                nc.sync.dma_start(w8_t[:, dp], w8t[:, dp])
                nc.sync.dma_start(x0[:, dp, :, :n0], x8t[:, dp, :, off0 : off0 + n0])
                nc.scalar.dma_start(rw8_t[:, dp], rw8t[:, dp])
                nc.scalar.dma_start(r0[:, dp, :, :n0], r8t[:, dp, :, off0 : off0 + n0])
            span_tiles[0] = (x0, r0)
            nxt = 1
            nc.scalar.dma_start(b_t[:], bt[:])
            nc.scalar.dma_start(w2_t[:], w2t[:])
            if len(x_spans) > 1:
                load_x_span(nc.scalar)  # span 1
            if len(x_spans) > 2:
                load_x_span(nc.scalar)  # span 2

            # Warm the ACT table during the startup DMA window so the
            # first real relu doesn't pay the ~1.5us table load.
            warm = cpool.tile([P, 1], f32)
            nc.any.memset(warm[:], 0.0)
            nc.scalar.activation(warm[:], warm[:], relu, bias=0.0)

            y_tile = None  # current [C, YGRAN] output staging tile
            y_base = 0

            def emit_l2(pend):
                # layer 2 for an already-relu'd chunk: y^T = W2^T h^T + b2
                nonlocal y_tile, y_base
                h_t, off, n = pend
                ps2 = pp2.tile([C, CHUNK], f32, name="ps2")
                for hb in range(HBLK):
                    nc.tensor.matmul(
                        ps2[:, :n],
                        w2_t[:, hb * C : (hb + 1) * C],
                        h_t[:, hb, :n],
                        start=(hb == 0),
                        stop=(hb == HBLK - 1),
                    )
                if y_tile is None:
                    y_tile = ypool.tile([C, YGRAN], f32, name="y_t")
                    y_base = off
                lo = off - y_base
                nc.vector.tensor_scalar(
                    y_tile[:, lo : lo + n],
                    ps2[:, :n],
                    scalar1=b_t[:C, HBLK : HBLK + 1],
                    scalar2=None,
                    op0=add,
                )
                if lo + n + CHUNK > YGRAN or off + n >= S:
                    # y stores ride the scalar queue: sync carries the big x
                    # span loads, and the final store must not queue behind
                    # one (the kernel cannot end before it completes).
                    nc.scalar.dma_start(
                        yt[:, y_base : y_base + lo + n], y_tile[:, : lo + n]
                    )
                    y_tile = None

            def l1_block(ps, x_t, r_t, dp, hb, o, n, first, last):
                # Three DoubleRow passes accumulating 16*(x @ W1) for one
                # (dp, hb) block.  P1/P2 share the W8 stationary (one LDW).
                w8b = w8_t[:, dp, :, hb * P : (hb + 1) * P]
                rw8b = rw8_t[:, dp, :, hb * P : (hb + 1) * P]
                xb = x_t[:, dp, :, o : o + n]
                rb = r_t[:, dp, :, o : o + n]
                nc.tensor.matmul(ps[:, :n], w8b, xb, start=first, stop=False, perf_mode=dr)
                nc.tensor.matmul(ps[:, :n], w8b, rb, start=False, stop=False, perf_mode=dr)
                nc.tensor.matmul(ps[:, :n], rw8b, xb, start=False, stop=last, perf_mode=dr)

            chunks = []  # (span_idx, global_off, local_off, n)
            for si, (soff, sn) in enumerate(x_spans):
                for o in range(0, sn, CHUNK):
                    chunks.append((si, soff + o, o, min(CHUNK, sn - o)))

            # Chunk 0 runs dp-outer so each arriving x dp-slice feeds all
            # three h-block accumulators immediately (DMA drip-feed).
            # Steady-state chunks run hb-outer/dp-inner so relu(hb) overlaps
            # the next h-block's matmul stream within the same chunk, and
            # layer-2 of chunk k-1 (emitted at the end of chunk k) has a
            # full chunk of slack behind relu(k-1, h2).
            pending = None
            for ci, (si, goff, o, n) in enumerate(chunks):
                x_t, r_t = span_tiles[si]
                if o == 0 and ci > 0 and nxt < len(x_spans):
                    load_x_span(nc.sync)
                h_t = hpool.tile([P, HBLK, CHUNK], bf16, name="h_t")
                if ci == 0:
                    pss = [pp1.tile([P, CHUNK], f32, name="ps") for _ in range(HBLK)]
                    for dp in range(DPAIR):
                        for hb in range(HBLK):
                            l1_block(
                                pss[hb], x_t, r_t, dp, hb, o, n,
                                first=(dp == 0), last=(dp == DPAIR - 1),
                            )
                    for hb in range(HBLK):
                        nc.scalar.activation(
                            h_t[:, hb, :n], pss[hb][:, :n], relu,
                            bias=b_t[:, hb : hb + 1], scale=1.0 / WSCALE,
                        )
                else:
                    for hb in range(HBLK):
                        ps = pp1.tile([P, CHUNK], f32, name="ps")
                        for dp in range(DPAIR):
                            l1_block(
                                ps, x_t, r_t, dp, hb, o, n,
                                first=(dp == 0), last=(dp == DPAIR - 1),
                            )
                        nc.scalar.activation(
                            h_t[:, hb, :n], ps[:, :n], relu,
                            bias=b_t[:, hb : hb + 1], scale=1.0 / WSCALE,
                        )
                if pending is not None:
                    emit_l2(pending)
                if o + n >= x_spans[si][1]:
                    span_tiles.pop(si, None)
                pending = (h_t, goff, n)
            emit_l2(pending)

    return _split_excess_waits(nc)


def kernel(x, W1, b1, W2, b2, question_types):
    global last_results
    x = np.ascontiguousarray(np.asarray(x, dtype=np.float32))
    W1 = np.asarray(W1, dtype=np.float32)
    b1 = np.asarray(b1, dtype=np.float32)
    W2 = np.asarray(W2, dtype=np.float32)
    b2 = np.asarray(b2, dtype=np.float32)
    qt = np.asarray(question_types)
    N = x.shape[0]

    idx = [np.nonzero(qt == e)[0] for e in range(E)]
    counts = [len(i) for i in idx]
    S = max(int(np.ceil(max(counts) / 16) * 16), 2 * CHUNK)

    nc = _program_cache.get(S)
    if nc is None:
        nc = _build_program(S)
        _program_cache[S] = nc

    in_maps = []
    for e in range(E):
        cnt = counts[e]
        xp = np.zeros((S, D), np.float32)
        xp[:cnt] = x[idx[e]]
        # [D, S] -> [P, DPAIR, 2, S]: d = dp*256 + t*128 + p
        xT = np.ascontiguousarray(xp.T).reshape(DPAIR, 2, P, S).transpose(2, 0, 1, 3)
        x8 = xT.astype(F8)
        r8 = (xT - x8.astype(np.float32)).astype(F8)
        w16 = (WSCALE * W1[e]).reshape(DPAIR, 2, P, H).transpose(2, 0, 1, 3)
        w8 = w16.astype(F8)
        rw8 = (w16 - w8.astype(np.float32)).astype(F8)
        w2 = (
            W2[e]
            .reshape(HBLK, P, C)
            .transpose(1, 0, 2)
            .reshape(P, HBLK * C)
            .astype(ml_dtypes.bfloat16)
        )
        btab = np.zeros((P, HBLK + 1), np.float32)
        btab[:, :HBLK] = b1[e].reshape(HBLK, P).T
        btab[:C, HBLK] = b2[e]
        in_maps.append(
            {
                "x8t": np.ascontiguousarray(x8),
                "r8t": np.ascontiguousarray(r8),
                "w8t": np.ascontiguousarray(w8),
                "rw8t": np.ascontiguousarray(rw8),
                "w2t": w2,
                "bt": btab,
            }
        )

    r = run_bass_kernel_spmd(nc, in_maps, list(range(NCORES)))
    last_results = r

    out = np.zeros((N, C), np.float32)
    for e in range(E):
        out[idx[e]] = r.results[e]["yt"][:, : counts[e]].T
    return out


# revision 18
# speedup vs baseline: 1.3053x; 1.3053x over previous
"""Routed MoE classifier head for Trainium2 (8 NeuronCores, SPMD).

The reference computes all 8 experts densely and selects; here each sample is
routed to exactly one expert.  On the host we gather samples by expert
(expert e -> core e), pad to a common S, and pre-transpose x so the
contraction dim D lands on SBUF partitions.  Each core runs a dense 2-layer
MLP (768 -> relu 384 -> 8) over its expert's samples:

  layer 1:  h^T = relu(W1^T x^T + b1)   as matmul(psum, lhsT=W1 [128,128],
            rhs=xT [128,n]) accumulated over 6 d-blocks per h-block
  layer 2:  y^T = W2^T h^T + b2

Matmul operands use float32r (fp32 bits, 1 column/cycle streaming — 4x the
fp32 paired-pass rate — with ~11-mantissa-bit operand rounding); PSUM
accumulation stays fp32.  Output y^T [8, S] is scattered back on the host.
"""

import ml_dtypes
import numpy as np

import concourse.bass as bass
import concourse.mybir as mybir
from concourse.tile import TileContext
from concourse.bass_utils import run_bass_kernel_spmd

P = 128
D = 768
H = 384
C = 8
E = 8
NCORES = 8
DBLK = D // P  # 6
HBLK = H // P  # 3
CHUNK = 512  # compute chunk (one PSUM bank of fp32)
XGRAN = 1536  # x DMA granularity (samples per load; multiple of CHUNK)
YGRAN = 2048  # y DMA granularity (samples per store)

MM_DTYPE = "bf16"

_program_cache = {}
last_results = None  # BassKernelResults of the most recent run (for test harness)


def _split_excess_waits(nc, max_waits=1):
    """The walrus build in this container only encodes one sem-wait per
    instruction; hoist extra waits onto NOPs inserted just before."""
    for blk in nc.main_func.blocks:
        insts = blk.instructions
        i = 0
        while i < len(insts):
            inst = insts[i]
            si = getattr(inst, "sync_info", None)
            if si is not None and si.on_wait and len(si.on_wait) > max_waits:
                waits = list(si.on_wait)
                extra, keep = waits[:-max_waits], waits[-max_waits:]
                nops = []
                for j in range(0, len(extra), max_waits):
                    nops.append(
                        mybir.InstNoOp(
                            name=f"{inst.name}-wsplit{j}",
                            engine=inst.engine,
                            bass_nofuse=True,
                            sync_info=mybir.SyncInfo(
                                on_wait=extra[j : j + max_waits], on_update=[]
                            ),
                        )
                    )
                inst.sync_info = mybir.SyncInfo(on_wait=keep, on_update=si.on_update)
                for k, nop in enumerate(nops):
                    nc.register_instruction(nop, overwrite=True)
                    insts.insert(i + k, nop)
                i += len(nops)
            i += 1
    return nc


def _spans2(total, lead, gran):
    """[(off, n), ...] covering `total`: leading spans from `lead`, then
    `gran`-sized spans (last one smaller)."""
    spans = []
    off = 0
    k = 0
    while off < total:
        n = min(lead[k] if k < len(lead) else gran, total - off)
        spans.append((off, n))
        off += n
        k += 1
    return spans


def _build_program(S):
    f32 = mybir.dt.float32
    fmm = {"f32r": mybir.dt.float32r, "bf16": mybir.dt.bfloat16}.get(MM_DTYPE, f32)
    relu = mybir.ActivationFunctionType.Relu
    add = mybir.AluOpType.add

    nc = bass.Bass(enable_partition_id=False)
    xt = nc.dram_tensor("xt", [P, DBLK, S], fmm, kind="ExternalInput")
    # w1 (6*384 cols) and w2 (3*8 cols) packed on the same 128 partitions
    wt = nc.dram_tensor("wt", [P, DBLK * H + HBLK * C], fmm, kind="ExternalInput")
    # b1 (3 cols, per h-block) and b2 (1 col, rows 0..7) packed
    bt = nc.dram_tensor("bt", [P, HBLK + 1], f32, kind="ExternalInput")
    yt = nc.dram_tensor("yt", [C, S], f32, kind="ExternalOutput")

    x_spans = _spans2(S, [CHUNK, CHUNK, CHUNK], XGRAN)

    with TileContext(nc) as tc:
        with (
            tc.tile_pool(name="const", bufs=1) as cpool,
            tc.tile_pool(name="xin", bufs=3) as xpool,
            tc.tile_pool(name="hbuf", bufs=3) as hpool,
            tc.tile_pool(name="yout", bufs=2) as ypool,
            tc.tile_pool(name="psum1", bufs=6, space="PSUM") as pp1,
            tc.tile_pool(name="psum2", bufs=2, space="PSUM") as pp2,
        ):
            # DMA descriptor generation costs the issuing engine ~0.6us of
            # queue time per dma_start, so steady-state x spans are ONE
            # descriptor each on the sync queue (which otherwise only does
            # y stores); the scalar queue stays clean for relus.  Startup:
            # x span 0 drips per-d-block on sync (chunk 0 is db-outer), W1
            # rides scalar as [db=0 | rest] so the first matmul only waits
            # for its own 96KB.
            b_t = cpool.tile([P, HBLK + 1], f32)
            w_t = cpool.tile([P, DBLK * H + HBLK * C], fmm)
            span_tiles = {}
            nxt = 0  # next span index to issue

            def load_x_span(eng):
                nonlocal nxt
                si = nxt
                nxt += 1
                off, n = x_spans[si]
                x_t = xpool.tile([P, DBLK, XGRAN], fmm, name="x_t")
                eng.dma_start(x_t[:, :, :n], xt[:, :, off : off + n])
                span_tiles[si] = x_t

            # Startup: all 16 DMA engines fair-share the in-flight
            # transfers, so the first matmul's operands must be (nearly)
            # alone in flight.  Order: W1[db0] + the six x0 d-block pieces
            # first (both queues), with W1[db1:] staged in two pieces timed
            # to chunk 0's db-outer consumption at the cold-clock pace.
            off0, n0 = x_spans[0]
            x0 = xpool.tile([P, DBLK, XGRAN], fmm, name="x_t")

            def w_piece(eng, lo, hi):
                eng.dma_start(w_t[:, lo * H : hi * H], wt[:, lo * H : hi * H])

            def x0_piece(eng, db):
                eng.dma_start(x0[:, db, :n0], xt[:, db, off0 : off0 + n0])

            w_piece(nc.sync, 0, 1)  # W1[db0]
            x0_piece(nc.sync, 0)
            x0_piece(nc.scalar, 1)
            w_piece(nc.scalar, 1, 3)  # W1[db1..2]
            x0_piece(nc.sync, 2)
            x0_piece(nc.scalar, 3)
            x0_piece(nc.sync, 4)
            x0_piece(nc.scalar, 5)
            nc.scalar.dma_start(b_t[:], bt[:])
            # W1[db3..5] + W2 tail
            nc.scalar.dma_start(w_t[:, 3 * H :], wt[:, 3 * H :])
            span_tiles[0] = x0
            nxt = 1
            if len(x_spans) > 1:
                load_x_span(nc.scalar)  # span 1
            if len(x_spans) > 2:
                load_x_span(nc.sync)  # span 2

            # Warm the ACT table during the startup DMA window so the
            # first real relu doesn't pay the ~1.5us table load.
            warm = cpool.tile([P, 1], f32)
            nc.any.memset(warm[:], 0.0)
            nc.scalar.activation(warm[:], warm[:], relu, bias=0.0)

            y_tile = None  # current [C, YGRAN] output staging tile
            y_base = 0

            def emit_l2(pend):
                # layer 2 for an already-relu'd chunk: y^T = W2^T h^T + b2
                nonlocal y_tile, y_base
                h_t, off, n = pend
                ps2 = pp2.tile([C, CHUNK], f32, name="ps2")
                for hb in range(HBLK):
                    nc.tensor.matmul(
                        ps2[:, :n],
                        w_t[:, DBLK * H + hb * C : DBLK * H + (hb + 1) * C],
                        h_t[:, hb, :n],
                        start=(hb == 0),
                        stop=(hb == HBLK - 1),
                    )
                if y_tile is None:
                    y_tile = ypool.tile([C, YGRAN], f32, name="y_t")
                    y_base = off
                lo = off - y_base
                nc.vector.tensor_scalar(
                    y_tile[:, lo : lo + n],
                    ps2[:, :n],
                    scalar1=b_t[:C, HBLK : HBLK + 1],
                    scalar2=None,
                    op0=add,
                )
                if lo + n + CHUNK > YGRAN or off + n >= S:
                    # y stores ride the sync queue, which is idle by the
                    # time the final (tail-critical) store is dispatched;
                    # the scalar queue is still draining the last relus.
                    nc.sync.dma_start(
                        yt[:, y_base : y_base + lo + n], y_tile[:, : lo + n]
                    )
                    y_tile = None

            chunks = []  # (span_idx, global_off, local_off, n)
            for si, (soff, sn) in enumerate(x_spans):
                for o in range(0, sn, CHUNK):
                    chunks.append((si, soff + o, o, min(CHUNK, sn - o)))

            # Chunk 0 runs db-outer so each arriving x d-block slice feeds
            # all three h-block accumulators immediately (DMA drip-feed).
            # Steady-state chunks run hb-outer/db-inner so relu(hb) overlaps
            # the next h-block's matmul stream within the same chunk, and
            # layer-2 of chunk k-1 (emitted at the end of chunk k) has a
            # full chunk of slack behind relu(k-1, h2).
            pending = None
            for ci, (si, goff, o, n) in enumerate(chunks):
                x_t = span_tiles[si]
                if o == 0 and ci > 0 and nxt < len(x_spans):
                    load_x_span(nc.sync)
                h_t = hpool.tile([P, HBLK, CHUNK], fmm, name="h_t")
                if ci == 0:
                    pss = [pp1.tile([P, CHUNK], f32, name="ps") for _ in range(HBLK)]
                    for db in range(DBLK):
                        for hb in range(HBLK):
                            nc.tensor.matmul(
                                pss[hb][:, :n],
                                w_t[:, db * H + hb * P : db * H + (hb + 1) * P],
                                x_t[:, db, o : o + n],
                                start=(db == 0),
                                stop=(db == DBLK - 1),
                            )
                    for hb in range(HBLK):
                        nc.scalar.activation(
                            h_t[:, hb, :n], pss[hb][:, :n], relu,
                            bias=b_t[:, hb : hb + 1],
                        )
                else:
                    for hb in range(HBLK):
                        ps = pp1.tile([P, CHUNK], f32, name="ps")
                        for db in range(DBLK):
                            nc.tensor.matmul(
                                ps[:, :n],
                                w_t[:, db * H + hb * P : db * H + (hb + 1) * P],
                                x_t[:, db, o : o + n],
                                start=(db == 0),
                                stop=(db == DBLK - 1),
                            )
                        nc.scalar.activation(
                            h_t[:, hb, :n], ps[:, :n], relu,
                            bias=b_t[:, hb : hb + 1],
                        )
                if pending is not None:
                    emit_l2(pending)
                if o + n >= x_spans[si][1]:
                    span_tiles.pop(si, None)
                pending = (h_t, goff, n)
            emit_l2(pending)

    return _split_excess_waits(nc)


def kernel(x, W1, b1, W2, b2, question_types):
    global last_results
    x = np.ascontiguousarray(np.asarray(x, dtype=np.float32))
    W1 = np.asarray(W1, dtype=np.float32)
    b1 = np.asarray(b1, dtype=np.float32)
    W2 = np.asarray(W2, dtype=np.float32)
    b2 = np.asarray(b2, dtype=np.float32)
    qt = np.asarray(question_types)
    N = x.shape[0]

    idx = [np.nonzero(qt == e)[0] for e in range(E)]
    counts = [len(i) for i in idx]
    S = max(int(np.ceil(max(counts) / 16) * 16), 2 * CHUNK)

    nc = _program_cache.get(S)
    if nc is None:
        nc = _build_program(S)
        _program_cache[S] = nc

    mmnp = {"f32r": np.float32, "bf16": ml_dtypes.bfloat16}.get(MM_DTYPE, np.float32)
    in_maps = []
    for e in range(E):
        cnt = counts[e]
        xp = np.zeros((S, D), mmnp)
        xp[:cnt] = x[idx[e]].astype(mmnp)
        xt = np.ascontiguousarray(xp.T.reshape(DBLK, P, S).transpose(1, 0, 2))
        w1t = W1[e].reshape(DBLK, P, H).transpose(1, 0, 2).reshape(P, DBLK * H)
        w2t = W2[e].reshape(HBLK, P, C).transpose(1, 0, 2).reshape(P, HBLK * C)
        wt = np.ascontiguousarray(np.concatenate([w1t, w2t], axis=1)).astype(mmnp)
        bt = np.zeros((P, HBLK + 1), np.float32)
        bt[:, :HBLK] = b1[e].reshape(HBLK, P).T
        bt[:C, HBLK] = b2[e]
        in_maps.append({"xt": xt, "wt": wt, "bt": bt})

    r = run_bass_kernel_spmd(nc, in_maps, list(range(NCORES)))
    last_results = r

    out = np.zeros((N, C), np.float32)
    for e in range(E):
        out[idx[e]] = r.results[e]["yt"][:, : counts[e]].T
    return out



# revision 19
# speedup vs baseline: 1.3423x; 1.0283x over previous
"""Routed MoE classifier head for Trainium2 (8 NeuronCores, SPMD).

The reference computes all 8 experts densely and selects; here each sample is
routed to exactly one expert.  On the host we gather samples by expert
(expert e -> core e), pad to a common S, and pre-transpose x so the
contraction dim D lands on SBUF partitions.  Each core runs a dense 2-layer
MLP (768 -> relu 384 -> 8) over its expert's samples:

  layer 1:  h^T = relu(W1^T x^T + b1)   as matmul(psum, lhsT=W1 [128,128],
            rhs=xT [128,n]) accumulated over 6 d-blocks per h-block
  layer 2:  y^T = W2^T h^T + b2

Matmul operands use bfloat16 (same 1 column/cycle PE stream rate as f32r on
TRN2, but half the HBM traffic — the kernel is otherwise DMA-bound on the x
stream); PSUM accumulation stays fp32.  Output y^T [8, S] is scattered back
on the host.
"""

import ml_dtypes
import numpy as np

import concourse.bass as bass
import concourse.mybir as mybir
from concourse.tile import TileContext
from concourse.bass_utils import run_bass_kernel_spmd

P = 128
D = 768
H = 384
C = 8
E = 8
NCORES = 8
DBLK = D // P  # 6
HBLK = H // P  # 3
CHUNK = 512  # compute chunk (one PSUM bank of fp32)
XGRAN = 768  # x DMA granularity (samples per load)
YGRAN = 2048  # y DMA granularity (samples per store)

MM_DTYPE = "bf16"

_program_cache = {}
last_results = None  # BassKernelResults of the most recent run (for test harness)


def _split_excess_waits(nc, max_waits=1):
    """The walrus build in this container only encodes one sem-wait per
    instruction; hoist extra waits onto NOPs inserted just before."""
    for blk in nc.main_func.blocks:
        insts = blk.instructions
        i = 0
        while i < len(insts):
            inst = insts[i]
            si = getattr(inst, "sync_info", None)
            if si is not None and si.on_wait and len(si.on_wait) > max_waits:
                waits = list(si.on_wait)
                extra, keep = waits[:-max_waits], waits[-max_waits:]
                nops = []
                for j in range(0, len(extra), max_waits):
                    nops.append(
                        mybir.InstNoOp(
                            name=f"{inst.name}-wsplit{j}",
                            engine=inst.engine,
                            bass_nofuse=True,
                            sync_info=mybir.SyncInfo(
                                on_wait=extra[j : j + max_waits], on_update=[]
                            ),
                        )
                    )
                inst.sync_info = mybir.SyncInfo(on_wait=keep, on_update=si.on_update)
                for k, nop in enumerate(nops):
                    nc.register_instruction(nop, overwrite=True)
                    insts.insert(i + k, nop)
                i += len(nops)
            i += 1
    return nc


def _spans2(total, lead, gran):
    """[(off, n), ...] covering `total`: leading spans from `lead`, then
    `gran`-sized spans (last one smaller)."""
    spans = []
    off = 0
    k = 0
    while off < total:
        n = min(lead[k] if k < len(lead) else gran, total - off)
        spans.append((off, n))
        off += n
        k += 1
    return spans


def _build_program(S):
    f32 = mybir.dt.float32
    fmm = {"f32r": mybir.dt.float32r, "bf16": mybir.dt.bfloat16}.get(MM_DTYPE, f32)
    relu = mybir.ActivationFunctionType.Relu
    add = mybir.AluOpType.add

    nc = bass.Bass(enable_partition_id=False)
    xt = nc.dram_tensor("xt", [P, DBLK, S], fmm, kind="ExternalInput")
    # w1 (6*384 cols) and w2 (3*8 cols) packed on the same 128 partitions
    wt = nc.dram_tensor("wt", [P, DBLK * H + HBLK * C], fmm, kind="ExternalInput")
    # b1 (3 cols, per h-block) and b2 (1 col, rows 0..7) packed
    bt = nc.dram_tensor("bt", [P, HBLK + 1], f32, kind="ExternalInput")
    yt = nc.dram_tensor("yt", [C, S], f32, kind="ExternalOutput")

    x_spans = _spans2(S, [CHUNK, CHUNK], XGRAN)

    with TileContext(nc) as tc:
        with (
            tc.tile_pool(name="const", bufs=1) as cpool,
            tc.tile_pool(name="xin", bufs=3) as xpool,
            tc.tile_pool(name="hbuf", bufs=3) as hpool,
            tc.tile_pool(name="yout", bufs=2) as ypool,
            tc.tile_pool(name="psum1", bufs=6, space="PSUM") as pp1,
            tc.tile_pool(name="psum2", bufs=2, space="PSUM") as pp2,
        ):
            w_t = cpool.tile([P, DBLK * H + HBLK * C], fmm)
            nc.sync.dma_start(w_t[:], wt[:])
            b_t = cpool.tile([P, HBLK + 1], f32)
            nc.scalar.dma_start(b_t[:], bt[:])

            # Warm the ACT table during the startup DMA window so the
            # first real relu doesn't pay the ~1.5us table load.
            warm = cpool.tile([P, 1], f32)
            nc.any.memset(warm[:], 0.0)
            nc.scalar.activation(warm[:], warm[:], relu, bias=0.0)

            span_tiles = {}

            def load_x(span_idx):
                off, n = x_spans[span_idx]
                x_t = xpool.tile([P, DBLK, XGRAN], fmm, name="x_t")
                if span_idx == 0:
                    # Per-d-block pieces so the first matmuls only wait on the
                    # first slice; alternate dispatch between the Sync
                    # and (idle) Scalar HWDGE paths so the ~700ns per-DMA
                    # dispatch cost doesn't serialize on one engine.
                    for db in range(DBLK):
                        eng = nc.sync if db % 2 == 0 else nc.scalar
                        eng.dma_start(x_t[:, db, :n], xt[:, db, off : off + n])
                else:
                    nc.sync.dma_start(x_t[:, :, :n], xt[:, :, off : off + n])
                span_tiles[span_idx] = x_t

            y_tile = None  # current [C, YGRAN] output staging tile
            y_base = 0

            def emit_l2(pend):
                # layer 2 for an already-relu'd chunk: y^T = W2^T h^T + b2
                nonlocal y_tile, y_base
                h_t, off, n = pend
                ps2 = pp2.tile([C, CHUNK], f32, name="ps2")
                for hb in range(HBLK):
                    nc.tensor.matmul(
                        ps2[:, :n],
                        w_t[:, DBLK * H + hb * C : DBLK * H + (hb + 1) * C],
                        h_t[:, hb, :n],
                        start=(hb == 0),
                        stop=(hb == HBLK - 1),
                    )
                if y_tile is None:
                    y_tile = ypool.tile([C, YGRAN], f32, name="y_t")
                    y_base = off
                lo = off - y_base
                nc.vector.tensor_scalar(
                    y_tile[:, lo : lo + n],
                    ps2[:, :n],
                    scalar1=b_t[:C, HBLK : HBLK + 1],
                    scalar2=None,
                    op0=add,
                )
                if lo + n + CHUNK > YGRAN or off + n >= S:
                    nc.sync.dma_start(yt[:, y_base : y_base + lo + n], y_tile[:, : lo + n])
                    y_tile = None

            # Software pipeline: emit layer-2 of chunk k-1 between layer-1 of
            # chunk k and k+1 so the PE never waits on the ACT-relu epilogue.
            load_x(0)
            pending = None
            for si, (soff, sn) in enumerate(x_spans):
                x_t = span_tiles.pop(si)
                for o in range(0, sn, CHUNK):
                    n = min(CHUNK, sn - o)
                    h_t = hpool.tile([P, HBLK, CHUNK], fmm, name="h_t")
                    if si == 0:
                        # db-outer: consume each arriving x d-block slice
                        # across all h-block accumulators immediately
                        pss = [
                            pp1.tile([P, CHUNK], f32, name="ps")
                            for _ in range(HBLK)
                        ]
                        for db in range(DBLK):
                            for hb in range(HBLK):
                                nc.tensor.matmul(
                                    pss[hb][:, :n],
                                    w_t[:, db * H + hb * P : db * H + (hb + 1) * P],
                                    x_t[:, db, o : o + n],
                                    start=(db == 0),
                                    stop=(db == DBLK - 1),
                                )
                        for hb in range(HBLK):
                            nc.scalar.activation(
                                h_t[:, hb, :n], pss[hb][:, :n], relu,
                                bias=b_t[:, hb : hb + 1],
                            )
                    else:
                        for hb in range(HBLK):
                            ps = pp1.tile([P, CHUNK], f32, name="ps")
                            for db in range(DBLK):
                                nc.tensor.matmul(
                                    ps[:, :n],
                                    w_t[:, db * H + hb * P : db * H + (hb + 1) * P],
                                    x_t[:, db, o : o + n],
                                    start=(db == 0),
                                    stop=(db == DBLK - 1),
                                )
                            nc.scalar.activation(
                                h_t[:, hb, :n], ps[:, :n], relu,
                                bias=b_t[:, hb : hb + 1],
                            )
                    if o == 0 and si + 1 < len(x_spans):
                        load_x(si + 1)
                    if pending is not None:
                        emit_l2(pending)
                    pending = (h_t, soff + o, n)
            emit_l2(pending)

    return _split_excess_waits(nc)


def kernel(x, W1, b1, W2, b2, question_types):
    global last_results
    x = np.ascontiguousarray(np.asarray(x, dtype=np.float32))
    W1 = np.asarray(W1, dtype=np.float32)
    b1 = np.asarray(b1, dtype=np.float32)
    W2 = np.asarray(W2, dtype=np.float32)
    b2 = np.asarray(b2, dtype=np.float32)
    qt = np.asarray(question_types)
    N = x.shape[0]

    idx = [np.nonzero(qt == e)[0] for e in range(E)]
    counts = [len(i) for i in idx]
    S = max(int(np.ceil(max(counts) / 16) * 16), 2 * CHUNK)

    nc = _program_cache.get(S)
    if nc is None:
        nc = _build_program(S)
        _program_cache[S] = nc

    mmnp = {"f32r": np.float32, "bf16": ml_dtypes.bfloat16}.get(MM_DTYPE, np.float32)
    in_maps = []
    for e in range(E):
        cnt = counts[e]
        xp = np.zeros((S, D), mmnp)
        xp[:cnt] = x[idx[e]].astype(mmnp)
        xt = np.ascontiguousarray(xp.T.reshape(DBLK, P, S).transpose(1, 0, 2))
        w1t = W1[e].reshape(DBLK, P, H).transpose(1, 0, 2).reshape(P, DBLK * H)
        w2t = W2[e].reshape(HBLK, P, C).transpose(1, 0, 2).reshape(P, HBLK * C)
        wt = np.ascontiguousarray(np.concatenate([w1t, w2t], axis=1)).astype(mmnp)
        bt = np.zeros((P, HBLK + 1), np.float32)
        bt[:, :HBLK] = b1[e].reshape(HBLK, P).T
        bt[:C, HBLK] = b2[e]
        in_maps.append({"xt": xt, "wt": wt, "bt": bt})

    r = run_bass_kernel_spmd(nc, in_maps, list(range(NCORES)))
    last_results = r

    out = np.zeros((N, C), np.float32)
    for e in range(E):
        out[idx[e]] = r.results[e]["yt"][:, : counts[e]].T
    return out
